# revision 1
# baseline (speedup 1.0000x reference)
"""CAM (channel-attention) kernel for Trainium2, data-parallel over batch on 8 cores.

Reference computation (per sample b):
    avg[c] = mean over spatial of x[b, c, :, :]
    mx[c]  = max  over spatial of x[b, c, :, :]
    gate   = sigmoid(W2 @ relu(W1 @ avg) + W2 @ relu(W1 @ mx))
    y[b]   = x[b] * gate[:, None, None]

Cost-model facts this schedule is built on (CoreSim / TRN2 spec):
  - A DMA occupies its issuing engine's queue for the whole transfer
    (~4.84 us per [128, 3136] f32 tile, ~332 GB/s per ring). Three rings
    exist: SP and ACT (HWDGE) and Pool (SWDGE via gpsimd). DMA and compute
    on the same engine serialize.
  - Only Pool DMAs can cast dtypes, and transfer time is charged on the
    OUTPUT bytes: an f32->f16 casting load costs ~2.3 us instead of 4.8.
  - DVE elementwise ops on all-SBUF f16 operands run in 4x mode
    (0.26 ns/elem); reductions always run 1x (1.04 ns/elem) regardless of
    dtype. A 4-level pairwise f16 max tree + small reduce costs ~2.0 us
    vs 3.3 us for a flat reduce.
  - ACT activation(Copy, accum_out) gives the per-channel sum in one
    ~3.0 us pass (the full-size out is a dummy write).

Schedule (per core, 4 samples x 4 channel-group tiles, f16 precision for
x; tolerance is 2e-2 so f16 rounding of x/stats is safe):
  - Pool ring: 16 casting loads (f32->f16) + 4 casting stores (f16->f32)
  - ACT: 16 sum-accum passes + sigmoid per sample + 1 mul + 1 store
  - DVE: 16 max trees + relu/hsum/g2 smalls + 11 f32-out muls + 4 f16-out muls
  - SP: 11 stores
  - PE: the tiny shared MLP (8 single-column accumulating matmuls for layer 1
    with separate sum/max stat tiles so each matmul has one producer,
    4 column matmuls for layer 2). relu(W1@sum)/S == relu(W1@avg) by
    positive homogeneity; the 1/S lands in the hsum combine.
"""

import numpy as np

import concourse.bacc as bacc
import concourse.bass as bass
import concourse.tile as tile
from concourse import mybir

N_CORES = 8
B = 32
C = 512
S = 56 * 56  # 3136
BPC = B // N_CORES  # samples per core
P = 128
CI = C // P  # channel groups of 128
HID = 32

F32 = mybir.dt.float32
F16 = mybir.dt.float16
AF = mybir.ActivationFunctionType

# max-tree halving widths (S = 3136)
TREE_W = (1568, 784, 392, 196)
H = 1568  # pair-add width for the sum split

# per-tile (t = 4*b + ci) mul engine: 'd' = DVE f32-out, 'h' = DVE f16-out
# (paired with a Pool casting store), 'a' = ACT f32-out, 'p' = Pool f32-out.
MUL_ENG = [
    "a", "h", "a", "h",   # b0
    "d", "h", "a", "d",   # b1
    "d", "h", "a", "a",   # b2
    "-", "-", "-", "-",   # b3 (handled by the tail block)
]
# pair-add tree depth for the sum per tile (0 = full-width ACT accum)
SUM_DEPTH = [
    2, 2, 0, 0,   # b0 (c2/c3 are the f32 fill tiles)
    2, 2, 2, 2,   # b1
    2, 2, 2, 2,   # b2
    2, 2, 2, 2,   # b3
]
# store engine per tile ('h' tiles must store via Pool casting DMA)
STORE_ENG = [
    "sp", "pool", "sp", "pool",  # b0
    "sp", "pool", "act", "sp",   # b1
    "sp", "pool", "sp", "sp",    # b2
    "-", "-", "-", "-",          # b3 (tail block)
]
# tail half-store ring per (ci, half): s=SP, a=ACT, p=Pool
TAIL_STORE = ["a", "p", "s", "a", "s", "p", "p", "a"]  # c1 halves (idx 1,5) must be "p"

LAST_RESULTS = None  # BassKernelResults of the most recent run (for test harness)
_NC_CACHE = None


def _build_bass():
    nc = bacc.Bacc()
    x = nc.dram_tensor("x", (BPC, P, CI, S), F32, kind="ExternalInput")
    w1t = nc.dram_tensor("w1t", (P, CI, HID), F32, kind="ExternalInput")
    w2t = nc.dram_tensor("w2t", (HID, 2 * C), F32, kind="ExternalInput")
    y = nc.dram_tensor("y", (BPC, CI, P, S), F32, kind="ExternalOutput")

    with tile.TileContext(nc) as tc:
        with (
            tc.tile_pool(name="xh", bufs=3) as xhp,        # f16 merged sample tiles
            tc.tile_pool(name="xh0", bufs=1) as xh0p,      # f16 first-sample tiles
            tc.tile_pool(name="x32", bufs=2) as x32p,      # f32 first-sample tiles
            tc.tile_pool(name="ysp", bufs=2) as yspp,      # f32 out tiles -> SP
            tc.tile_pool(name="y16", bufs=2) as y16p,      # f16 out tiles -> Pool
            tc.tile_pool(name="ypl", bufs=1) as yplp,      # f32 out tiles (Pool mul)
            tc.tile_pool(name="tree", bufs=1) as treep,    # max-tree scratch
            tc.tile_pool(name="h1p", bufs=1) as h1p,       # pair-add sum scratch
            tc.tile_pool(name="consts", bufs=1) as consts,
            tc.tile_pool(name="small", bufs=4) as small,
            tc.tile_pool(name="dump", bufs=1) as dump,
            tc.tile_pool(name="ps1", bufs=2, space=bass.MemorySpace.PSUM) as ps1,
            tc.tile_pool(name="ps2", bufs=2, space=bass.MemorySpace.PSUM) as ps2,
            tc.tile_pool(name="psw", bufs=1, space=bass.MemorySpace.PSUM) as psw,
        ):
            w1t_sb = consts.tile([P, CI, HID], F32)
            nc.sync.dma_start(out=w1t_sb[:], in_=w1t[:])
            w2t_sb = consts.tile([HID, 2 * C], F32)
            nc.sync.dma_start(out=w2t_sb[:], in_=w2t[:])
            zeros = consts.tile([P, CI], F32)
            nc.vector.memset(zeros[:], 0.0)
            # f16 copy of W1^T for the wide-hidden matmuls (moving is f16)
            w1t16_sb = consts.tile([P, CI, HID], F16)
            nc.vector.tensor_copy(out=w1t16_sb[:], in_=w1t_sb[:])

            # PE observes the weight-DMA/cast semaphores here, once.
            pw = ps1.tile([HID, 1], F32, tag="p1", name="pw")
            nc.tensor.matmul(pw[:], w1t_sb[:, 0, :], w1t_sb[:, 0, 0:1])
            pw2 = ps2.tile([P, CI], F32, tag="p2", name="pw2")
            nc.tensor.matmul(pw2[:, 0:1], w2t_sb[:, 0:P], w2t_sb[:, 0:1])
            pww = psw.tile([HID, H], F32, tag="p1w", name="pww")
            nc.tensor.matmul(pww[:, 0:1], w1t16_sb[:, 0, :],
                             w1t16_sb[:, 0, 0:1])

            # ---- loads. First sample split across rings so stats start fast:
            # b0c0/b0c1 casting loads on Pool, b0c2/b0c3 f32 on SP. Samples
            # b1..b3 stream as per-sample MERGED casting loads on Pool (one
            # DMA per sample saves the per-DMA ring overhead; the casting
            # stores are appended behind them later).
            xts = {}
            xsamp = {}
            F32_TILES = {(0, 2), (0, 3)}
            # b0c0 in two halves so the DVE tree can start ~2 us earlier
            t = xh0p.tile([P, S], F16, tag="xh0", name="xh0_0")
            nc.gpsimd.dma_start(out=t[:, 0:H], in_=x[0, :, 0, 0:H])
            nc.gpsimd.dma_start(out=t[:, H:S], in_=x[0, :, 0, H:S])
            xts[(0, 0)] = t
            t = xh0p.tile([P, S], F16, tag="xh1", name="xh0_1")
            nc.gpsimd.dma_start(out=t[:], in_=x[0, :, 1, :])
            xts[(0, 1)] = t
            for ci in range(2, CI):
                t = x32p.tile([P, S], F32, tag="x32", name=f"x32_{ci}")
                nc.sync.dma_start(out=t[:], in_=x[0, :, ci, :])
                xts[(0, ci)] = t

            def emit_load(b):
                ts = xhp.tile([P, CI, S], F16, tag="xhs", name=f"xhs{b}")
                if (b, 0) in F32_TILES:
                    # c0 rides SP as f32 (SP is idle early); Pool casts c1..c3
                    t32 = x32p.tile([P, S], F32, tag="x32", name=f"x32_{b}_0")
                    nc.sync.dma_start(out=t32[:], in_=x[b, :, 0, :])
                    xts[(b, 0)] = t32
                    nc.gpsimd.dma_start(out=ts[:, 1:CI, :], in_=x[b, :, 1:CI, :])
                else:
                    nc.gpsimd.dma_start(out=ts[:], in_=x[b])
                xsamp[b] = ts

            def xap(b, ci, lo=0, hi=S):
                if (b, ci) in xts:
                    return xts[(b, ci)][:, lo:hi]
                return xsamp[b][:, ci, lo:hi]

            stats_t, g_t = {}, {}

            p1w_t, p1s_t = {}, {}
            # chunk boundaries inside the [HID, H] wide-hidden PSUM
            # accumulator (each matmul output stays inside one PSUM bank)
            CHUNKS = [(0, 512), (512, 1024), (1024, 1536), (1536, H)]

            def emit_stats(b, ci, stats):
                """DVE: pair-add + max tree; PE: W1 @ h1 into the wide psum.

                The per-channel sums never materialize: W1 is linear, so
                p1_sum = W1 @ (sum_j x_j) = sum_j columns of W1 @ h1, which
                accumulates in PSUM across the chunk matmuls; one tiny ACT
                accum pass then finishes the spatial reduction in hidden
                space ([HID, H] instead of [P, S]).
                """
                p1w = p1w_t[b]
                if (b, ci) in F32_TILES:
                    # f32 tiles: PE f32 matmuls are 4x slower, so give
                    # PE an f16 pair-add result instead (DVE 2x copy-add)
                    h1 = h1p.tile([P, H], F16, tag="h1", name=f"h1_{b}_{ci}")
                    nc.vector.tensor_tensor(
                        out=h1[:], in0=xap(b, ci, 0, H), in1=xap(b, ci, H, S),
                        op=mybir.AluOpType.add,
                    )
                    for (lo, hi) in CHUNKS:
                        nc.tensor.matmul(
                            p1w[:, lo:hi],
                            w1t16_sb[:, ci, :],
                            h1[:, lo:hi],
                            start=(ci == 0),
                            stop=(ci == CI - 1),
                            skip_group_check=True,
                        )
                else:
                    # PE does the spatial fold itself: accumulate
                    # W1 @ x[:, lo:hi] + W1 @ x[:, H+lo:H+hi] per chunk
                    for (lo, hi) in CHUNKS:
                        nc.tensor.matmul(
                            p1w[:, lo:hi],
                            w1t16_sb[:, ci, :],
                            xap(b, ci, lo, hi),
                            start=(ci == 0),
                            stop=False,
                            skip_group_check=True,
                        )
                        nc.tensor.matmul(
                            p1w[:, lo:hi],
                            w1t16_sb[:, ci, :],
                            xap(b, ci, H + lo, H + hi),
                            start=False,
                            stop=(ci == CI - 1),
                            skip_group_check=True,
                        )
                # DVE: pairwise f16 max tree, then a small flat reduce
                src = None
                for li, w in enumerate(TREE_W):
                    tt = treep.tile([P, w], F16, tag=f"tr{li}", name=f"tr{b}_{ci}_{li}")
                    if li == 0:
                        in0, in1 = xap(b, ci, 0, w), xap(b, ci, w, 2 * w)
                    else:
                        in0, in1 = src[:, 0:w], src[:, w : 2 * w]
                    nc.vector.tensor_tensor(
                        out=tt[:], in0=in0, in1=in1, op=mybir.AluOpType.max,
                    )
                    src = tt
                nc.vector.reduce_max(
                    out=stats[:, ci : ci + 1],
                    in_=src[:],
                    axis=mybir.AxisListType.X,
                )

            p1m_t, p2_t, hsum_t = {}, {}, {}

            def emit_p1m(b, stats):
                """PE: max-branch layer-1 (4 accumulating [32,1] matmuls)."""
                p1m = ps1.tile([HID, 1], F32, tag="p1", name=f"p1m_{b}")
                for ci in range(CI):
                    nc.tensor.matmul(
                        p1m[:],
                        w1t_sb[:, ci, :],
                        stats[:, ci : ci + 1],
                        start=(ci == 0),
                        stop=(ci == CI - 1),
                    )
                p1m_t[b] = p1m

            def emit_p1sum_accum(b):
                """ACT: finish the sum branch: accum over p1w [HID, H] psum."""
                p1s = small.tile([HID, 1], F32, tag="p1s", name=f"p1s_{b}")
                dmy = dump.tile([HID, H], F16, tag="dmy", name=f"pdmy{b}")
                nc.scalar.activation(
                    out=dmy[:],
                    in_=p1w_t[b][:],
                    func=AF.Copy,
                    accum_out=p1s[:],
                )
                p1s_t[b] = p1s

            def emit_relu_hsum(b):
                """DVE relus into h[:,0]=sum-branch, h[:,1]=max-branch."""
                h = small.tile([HID, 2], F32, tag="h", name=f"h{b}")
                nc.vector.tensor_scalar_max(out=h[:, 0:1], in0=p1s_t[b][:],
                                            scalar1=0.0)
                nc.vector.tensor_scalar_max(out=h[:, 1:2], in0=p1m_t[b][:],
                                            scalar1=0.0)
                hsum_t[b] = h

            def emit_mlp_l2(b):
                """PE layer-2: p2 = (W2/S) @ relu_h0 + W2 @ relu_h1."""
                h = hsum_t[b]
                p2 = ps2.tile([P, CI], F32, tag="p2", name=f"p2_{b}")
                nc.tensor.matmul(
                    p2[:], w2t_sb[:, 0:P], zeros[:HID, 0:CI],
                    start=True, stop=False, skip_group_check=True,
                )
                for ci in range(CI):
                    nc.tensor.matmul(
                        p2[:, ci : ci + 1],
                        w2t_sb[:, C + ci * P : C + (ci + 1) * P],
                        h[:, 0:1],
                        start=False, stop=False, skip_group_check=True,
                    )
                    nc.tensor.matmul(
                        p2[:, ci : ci + 1],
                        w2t_sb[:, ci * P : (ci + 1) * P],
                        h[:, 1:2],
                        start=False, stop=(ci == CI - 1), skip_group_check=True,
                    )
                p2_t[b] = p2

            def emit_sigmoid(b):
                g = small.tile([P, CI], F32, tag="g", name=f"g{b}")
                nc.scalar.activation(out=g[:], in_=p2_t[b][:], func=AF.Sigmoid)
                g_t[b] = g

            def emit_mul_store(b, ci):
                t = 4 * b + ci
                xt = xap(b, ci)
                m, st = MUL_ENG[t], STORE_ENG[t]
                g = g_t[b]
                if m == "h":
                    yt = y16p.tile([P, S], F16, tag="y16", name=f"y16_{b}_{ci}")
                    nc.vector.tensor_scalar_mul(
                        out=yt[:], in0=xt, scalar1=g[:, ci : ci + 1]
                    )
                elif m == "a":
                    yt = yspp.tile([P, S], F32, tag="yota", name=f"yot_{b}_{ci}")
                    nc.scalar.activation(
                        out=yt[:], in_=xt, func=AF.Copy,
                        scale=g[:, ci : ci + 1],
                    )
                elif m == "p":
                    yt = yplp.tile([P, S], F32, tag="ypl", name=f"ypl_{b}_{ci}")
                    nc.gpsimd.tensor_scalar_mul(
                        out=yt[:], in0=xt, scalar1=g[:, ci : ci + 1]
                    )
                else:
                    yt = yspp.tile([P, S], F32, tag="ysp", name=f"ysp_{b}_{ci}")
                    nc.vector.tensor_scalar_mul(
                        out=yt[:], in0=xt, scalar1=g[:, ci : ci + 1]
                    )
                eng = {"sp": nc.sync, "act": nc.scalar, "pool": nc.gpsimd}[st]
                eng.dma_start(out=y[b, ci], in_=yt[:])

            # ---- software pipeline: iteration k computes stats(k) and the
            # mul/store of sample k-1 (whose gate chain runs during stats(k)).
            # Per-engine queue order is what matters: MLP stages that wait on
            # cross-engine sems are sandwiched behind stats work that is
            # already runnable, so no engine stalls at its queue head.
            for k in range(BPC):
                if k + 1 < BPC:
                    emit_load(k + 1)        # Pool queue head: always runnable
                stats = small.tile([P, CI], F32, tag="stats", name=f"st{k}")
                stats_t[k] = stats
                p1w_t[k] = psw.tile([HID, H], F32, tag="p1w", name=f"p1w_{k}")
                if k >= 1:
                    emit_p1m(k - 1, stats_t[k - 1])   # PE
                    emit_p1sum_accum(k - 1)           # ACT (tiny)
                    emit_relu_hsum(k - 1)   # DVE first: shortens gate chain
                    emit_mlp_l2(k - 1)      # PE
                    emit_sigmoid(k - 1)     # ACT
                emit_stats(k, 0, stats)
                if k >= 1:
                    emit_mul_store(k - 1, 0)
                    emit_mul_store(k - 1, 1)
                emit_stats(k, 1, stats)
                emit_stats(k, 2, stats)
                if k >= 1:
                    emit_mul_store(k - 1, 2)
                emit_stats(k, 3, stats)
                if k >= 1:
                    emit_mul_store(k - 1, 3)
            # ---- tail: sample BPC-1. Muls full-tile across engines, but
            # stores split into half-tile DMAs spread over all three rings.
            bl = BPC - 1
            emit_p1m(bl, stats_t[bl])
            emit_p1sum_accum(bl)
            emit_relu_hsum(bl)
            emit_mlp_l2(bl)
            emit_sigmoid(bl)
            g = g_t[bl]
            # muls: c0 full on Pool; c1/c2/c3 in HALF tiles on DVE (all
            # f32 out so every ring can store them); halves spread per
            # TAIL_STORE so the rings drain in parallel right after the gate.
            HH = S // 2
            ypl = yplp.tile([P, S], F32, tag="ypl", name="ytl_0")
            y1 = y16p.tile([P, S], F16, tag="y16", name="ytl_1")
            y2 = yspp.tile([P, S], F32, tag="ysp", name="ytl_2")
            y3 = yspp.tile([P, S], F32, tag="ysp", name="ytl_3")
            for ci, yt, lo, hi in [
                (1, y1, 0, HH), (2, y2, 0, HH), (0, ypl, 0, HH), (3, y3, 0, HH),
                (1, y1, HH, S), (2, y2, HH, S), (0, ypl, HH, S), (3, y3, HH, S),
            ]:
                nc.vector.tensor_scalar_mul(
                    out=yt[:, lo:hi], in0=xap(bl, ci, lo, hi),
                    scalar1=g[:, ci : ci + 1])
            emap = {"s": nc.sync, "a": nc.scalar, "p": nc.gpsimd}
            srcs = {0: ypl, 1: y1, 2: y2, 3: y3}
            order = [(2, 0), (1, 0), (3, 0), (0, 0), (2, 1), (1, 1), (3, 1), (0, 1)]
            for i, (ci, half) in enumerate(order):
                lo, hi = half * HH, (half + 1) * HH
                emap[TAIL_STORE[i]].dma_start(out=y[bl, ci, :, lo:hi],
                                              in_=srcs[ci][:, lo:hi])
    nc.compile()
    return nc


_RUNNER = None


def _make_runner(nc):
    """jit(shard_map) over the bass_exec custom call — the same lowering
    run_bass_kernel_spmd uses under axon, but built once and cached so
    repeated kernel() calls reuse one loaded executable (loading a second
    copy of the NEFF in the same process wedges the device)."""
    import jax
    from jax.sharding import Mesh, PartitionSpec
    from jax.experimental.shard_map import shard_map
    from concourse.bass2jax import (
        _bass_exec_p,
        install_neuronx_cc_hook,
        partition_id_tensor,
    )

    install_neuronx_cc_hook()
    partition_name = nc.partition_id_tensor.name if nc.partition_id_tensor else None
    in_names, out_names, out_avals = [], [], []
    for alloc in nc.m.functions[0].allocations:
        if not isinstance(alloc, mybir.MemoryLocationSet):
            continue
        name = alloc.memorylocations[0].name
        if alloc.kind == "ExternalInput":
            if name != partition_name:
                in_names.append(name)
        elif alloc.kind == "ExternalOutput":
            out_names.append(name)
            out_avals.append(
                jax.core.ShapedArray(
                    tuple(alloc.tensor_shape), mybir.dt.np(alloc.dtype)
                )
            )
    all_in = in_names + out_names
    if partition_name is not None:
        all_in.append(partition_name)

    def _body(*args):
        operands = list(args)
        if partition_name is not None:
            operands.append(partition_id_tensor())
        outs = _bass_exec_p.bind(
            *operands,
            out_avals=tuple(out_avals),
            in_names=tuple(all_in),
            out_names=tuple(out_names),
            lowering_input_output_aliases=(),
            sim_require_finite=True,
            sim_require_nnan=True,
            nc=nc,
        )
        return tuple(outs)

    devices = jax.devices()[:N_CORES]
    mesh = Mesh(np.asarray(devices), ("core",))
    n_args = len(in_names) + len(out_names)
    fn = jax.jit(
        shard_map(
            _body,
            mesh=mesh,
            in_specs=(PartitionSpec("core"),) * n_args,
            out_specs=(PartitionSpec("core"),) * len(out_names),
            check_rep=False,
        ),
        keep_unused=True,
    )
    assert in_names == ["x", "w1t", "w2t"] and out_names == ["y"], (
        in_names,
        out_names,
    )
    return fn


def kernel(x, w1, w2, **_ignored):
    global _NC_CACHE, _RUNNER
    x = np.ascontiguousarray(np.asarray(x, dtype=np.float32))
    w1 = np.asarray(w1, dtype=np.float32)  # [HID, C]
    w2 = np.asarray(w2, dtype=np.float32)  # [C, HID]

    # SBUF layouts, pretransposed on host
    w1t = np.ascontiguousarray(
        w1.T.reshape(CI, P, HID).transpose(1, 0, 2)
    )  # [P, CI, HID]; w1t[p, ci, h] = w1[h, ci*128+p]
    w2t = np.ascontiguousarray(
        np.concatenate([w2.T, w2.T / S], axis=1)
    )  # [HID, 2*C]: plain W2^T cols 0:C, W2^T/S cols C:2C (sum branch)

    if _NC_CACHE is None:
        _NC_CACHE = _build_bass()
    if _RUNNER is None:
        _RUNNER = _make_runner(_NC_CACHE)

    # global inputs concatenated on axis 0; shard_map hands each core its
    # slice. x goes device-side as [b, p, ci, s] so one DMA can pull a whole
    # sample into a [P, CI, S] tile.
    xs = np.ascontiguousarray(
        x.reshape(N_CORES * BPC, CI, P, S).transpose(0, 2, 1, 3)
    )
    w1ts = np.concatenate([w1t] * N_CORES, axis=0)
    w2ts = np.concatenate([w2t] * N_CORES, axis=0)
    ybuf = np.zeros((N_CORES * BPC, CI, P, S), np.float32)
    (y,) = _RUNNER(xs, w1ts, w2ts, ybuf)
    return np.asarray(y).reshape(B, C, 56, 56)

